# revision 1
# baseline (speedup 1.0000x reference)
import math
import sys

sys.path.insert(0, "/opt/trn_rl_repo")

import numpy as np

# ---- model constants (from the reference nn.Module) ----
ROPE_PERIOD = 19.0
OMEGA = 2.0 * math.pi / ROPE_PERIOD
PEAK_EPS = 0.3
TARGET_LOGIT_GAP = math.log(10.0)
ATTN_AMPLITUDE = TARGET_LOGIT_GAP / (
    math.cos(OMEGA * PEAK_EPS) - math.cos(OMEGA * (1.0 - PEAK_EPS))
)
QK_NORM_SCALE = math.sqrt(ATTN_AMPLITUDE / math.sqrt(2.0))
SCALE = 2.0 ** (-0.5) * QK_NORM_SCALE**2
EMBED_CONST = 1000.0
EPS = 1e-6

B, L = 4, 4096
N_CORES = 8
NKB = 16          # key blocks of 128 per core (even or odd global blocks)
NQC = 8           # query chunks of 512
SQRT2 = math.sqrt(2.0)

_compiled = None


def _build():
    import concourse.bass as bass
    import concourse.tile as tile
    from concourse import bacc, mybir

    f32 = mybir.dt.float32
    AF = mybir.ActivationFunctionType
    OP = mybir.AluOpType

    nc = bacc.Bacc("TRN2", target_bir_lowering=False, debug=False,
                   num_devices=N_CORES)

    # per-core inputs (host pre-arranged layouts; pure indexing, no math)
    xq_d = nc.dram_tensor("xq", [128, 64], f32, kind="ExternalInput").ap()
    tq_d = nc.dram_tensor("tq", [128, 64], f32, kind="ExternalInput").ap()
    xkr_d = nc.dram_tensor("xkr", [128, 32], f32, kind="ExternalInput").ap()
    tkr_d = nc.dram_tensor("tkr", [128, 32], f32, kind="ExternalInput").ap()
    xkb_d = nc.dram_tensor("xkb", [128, 32], f32, kind="ExternalInput").ap()
    mask_d = nc.dram_tensor("masks", [128, 1024], f32, kind="ExternalInput").ap()
    wq_d = nc.dram_tensor("wq", [1], f32, kind="ExternalInput").ap()
    wv_d = nc.dram_tensor("wv", [1], f32, kind="ExternalInput").ap()
    wg_d = nc.dram_tensor("wg", [2], f32, kind="ExternalInput").ap()
    wc_d = nc.dram_tensor("wc", [1], f32, kind="ExternalInput").ap()
    out_d = nc.dram_tensor("out", [128, 64], f32, kind="ExternalOutput").ap()

    with tile.TileContext(nc) as tc:
        with (
            tc.tile_pool(name="const", bufs=1) as cp,
            tc.tile_pool(name="work", bufs=2) as wp,
            tc.tile_pool(name="ep", bufs=3) as ep,
            tc.tile_pool(name="pslog", bufs=2, space="PSUM") as pslog,
            tc.tile_pool(name="psnd", bufs=2, space="PSUM") as psnd,
            tc.tile_pool(name="psbc", bufs=1, space="PSUM") as psbc,
            tc.tile_pool(name="dram", bufs=1, space="DRAM") as dp,
        ):
            # ---------- load inputs ----------
            xq = cp.tile([128, 64], f32, tag="xq")
            nc.sync.dma_start(xq[:], xq_d[:])
            tq = cp.tile([128, 64], f32, tag="tq")
            nc.sync.dma_start(tq[:], tq_d[:])
            xkr = cp.tile([128, 32], f32, tag="xkr")
            nc.sync.dma_start(xkr[:], xkr_d[:])
            tkr = cp.tile([128, 32], f32, tag="tkr")
            nc.sync.dma_start(tkr[:], tkr_d[:])
            xkb = cp.tile([128, 32], f32, tag="xkb")
            nc.sync.dma_start(xkb[:], xkb_d[:])
            masks = cp.tile([128, 1024], f32, tag="masks")
            nc.sync.dma_start(masks[:], mask_d[:])

            sw = cp.tile([1, 8], f32, tag="sw")  # scalar workspace row
            nc.sync.dma_start(sw[0:1, 0:1], wq_d[0:1])
            nc.sync.dma_start(sw[0:1, 1:2], wv_d[0:1])
            nc.sync.dma_start(sw[0:1, 2:4], wg_d[0:2])
            nc.sync.dma_start(sw[0:1, 4:5], wc_d[0:1])

            # ---------- scalar prep: cos/sin(phi) by Taylor, gate consts ----
            # layout of wvec [1, 8]: 0=cS 1=sS 2=wv 3=ga 4=gc 5=ga2 6=wc 7=unused
            wvec = cp.tile([1, 8], f32, tag="wvec")
            t2 = cp.tile([1, 1], f32, tag="t2")
            nc.scalar.activation(t2[:], sw[0:1, 0:1], AF.Square)
            u = cp.tile([1, 1], f32, tag="u")
            nc.vector.tensor_scalar(u[:], t2[:], -1.0 / 720.0, 1.0 / 24.0,
                                    OP.mult, OP.add)
            nc.vector.tensor_scalar(u[:], u[:], t2[0:1, 0:1], -0.5,
                                    OP.mult, OP.add)
            nc.vector.tensor_scalar(u[:], u[:], t2[0:1, 0:1], 1.0,
                                    OP.mult, OP.add)
            nc.vector.tensor_scalar(wvec[0:1, 0:1], u[:], SCALE, None, OP.mult)
            v = cp.tile([1, 1], f32, tag="v")
            nc.vector.tensor_scalar(v[:], t2[:], -1.0 / 5040.0, 1.0 / 120.0,
                                    OP.mult, OP.add)
            nc.vector.tensor_scalar(v[:], v[:], t2[0:1, 0:1], -1.0 / 6.0,
                                    OP.mult, OP.add)
            nc.vector.tensor_scalar(v[:], v[:], t2[0:1, 0:1], 1.0,
                                    OP.mult, OP.add)
            nc.vector.tensor_scalar(v[:], v[:], sw[0:1, 0:1], None, OP.mult)
            nc.vector.tensor_scalar(wvec[0:1, 1:2], v[:], SCALE, None, OP.mult)
            nc.vector.tensor_copy(wvec[0:1, 2:3], sw[0:1, 1:2])
            nc.vector.tensor_copy(wvec[0:1, 3:4], sw[0:1, 2:3])
            nc.vector.tensor_copy(wvec[0:1, 4:5], sw[0:1, 3:4])
            nc.vector.tensor_scalar(wvec[0:1, 5:6], sw[0:1, 3:4],
                                    -1.0 / EMBED_CONST, sw[0:1, 2:3],
                                    OP.mult, OP.add)
            nc.vector.tensor_copy(wvec[0:1, 6:7], sw[0:1, 4:5])
            nc.vector.tensor_scalar(wvec[0:1, 7:8], sw[0:1, 4:5], 0.0, None,
                                    OP.mult)

            # broadcast scalars to all 128 partitions via K=1 matmul
            ones_row = cp.tile([1, 128], f32, tag="ones_row")
            nc.gpsimd.memset(ones_row[:], 1.0)
            bc_ps = psbc.tile([128, 8], f32)
            nc.tensor.matmul(bc_ps[:], ones_row[:], wvec[:], start=True,
                             stop=True)
            bc = cp.tile([128, 8], f32, tag="bc")
            nc.vector.tensor_copy(bc[:], bc_ps[:])

            def rms_r(x2sum, tag):
                # x2sum = sum of squares*0.5+eps -> returns rsqrt tile
                ln = wp.tile(x2sum.shape, f32, tag=tag + "_ln")
                nc.scalar.activation(ln[:], x2sum[:], AF.Ln)
                r = wp.tile(x2sum.shape, f32, tag=tag + "_r")
                nc.scalar.activation(r[:], ln[:], AF.Exp, scale=-0.5)
                return r

            # ---------- query pipeline ([128,32], pos = 32p+n) ----------
            x0, x1 = xq[:, 0:32], xq[:, 32:64]
            cosq, sinq = tq[:, 0:32], tq[:, 32:64]
            sq = wp.tile([128, 32], f32, tag="sq")
            nc.vector.tensor_tensor(sq[:], x0, x0, OP.mult)
            sq1 = wp.tile([128, 32], f32, tag="sq1")
            nc.vector.tensor_tensor(sq1[:], x1, x1, OP.mult)
            mq = wp.tile([128, 32], f32, tag="mq")
            nc.vector.tensor_tensor(mq[:], sq[:], sq1[:], OP.add)
            nc.vector.tensor_scalar(mq[:], mq[:], 0.5, EPS, OP.mult, OP.add)
            rq = rms_r(mq, "rq")
            xn0 = wp.tile([128, 32], f32, tag="xn0")
            nc.vector.tensor_tensor(xn0[:], x0, rq[:], OP.mult)
            am = wp.tile([128, 32], f32, tag="am")
            nc.vector.tensor_tensor(am[:], xn0[:], xn0[:], OP.mult)
            nc.vector.tensor_scalar(am[:], am[:], 0.5, EPS, OP.mult, OP.add)
            ra = rms_r(am, "ra")
            aq = wp.tile([128, 32], f32, tag="aq")
            nc.vector.tensor_tensor(aq[:], xn0[:], ra[:], OP.mult)
            qz = wp.tile([128, 32], f32, tag="qz")
            nc.scalar.activation(qz[:], aq[:], AF.Abs)
            nc.vector.tensor_scalar(qz[:], qz[:], -SCALE * SQRT2, None, OP.mult)
            t1 = wp.tile([128, 32], f32, tag="t1")
            nc.vector.tensor_scalar(t1[:], cosq, bc[:, 0:1], None, OP.mult)
            t2q = wp.tile([128, 32], f32, tag="t2q")
            nc.vector.tensor_scalar(t2q[:], sinq, bc[:, 1:2], None, OP.mult)
            nc.vector.tensor_tensor(t1[:], t1[:], t2q[:], OP.add)
            qx = wp.tile([128, 32], f32, tag="qx")
            nc.vector.tensor_tensor(qx[:], aq[:], t1[:], OP.mult)
            t3 = wp.tile([128, 32], f32, tag="t3")
            nc.vector.tensor_scalar(t3[:], sinq, bc[:, 0:1], None, OP.mult)
            t4 = wp.tile([128, 32], f32, tag="t4")
            nc.vector.tensor_scalar(t4[:], cosq, bc[:, 1:2], None, OP.mult)
            nc.vector.tensor_tensor(t3[:], t3[:], t4[:], OP.subtract)
            qy = wp.tile([128, 32], f32, tag="qy")
            nc.vector.tensor_tensor(qy[:], aq[:], t3[:], OP.mult)

            Qop = cp.tile([3, 4096], f32, tag="Qop")
            nc.sync.dma_start(Qop[0:1, :], qx[:])
            nc.sync.dma_start(Qop[1:2, :], qy[:])
            nc.sync.dma_start(Qop[2:3, :], qz[:])

            # ---------- key pipeline row-major ([128,16], t = 16p+n) ------
            k0, k1 = xkr[:, 0:16], xkr[:, 16:32]
            cosk, sink = tkr[:, 0:16], tkr[:, 16:32]
            ksq = wp.tile([128, 16], f32, tag="ksq")
            nc.vector.tensor_tensor(ksq[:], k0, k0, OP.mult)
            ksq1 = wp.tile([128, 16], f32, tag="ksq1")
            nc.vector.tensor_tensor(ksq1[:], k1, k1, OP.mult)
            mk = wp.tile([128, 16], f32, tag="mk")
            nc.vector.tensor_tensor(mk[:], ksq[:], ksq1[:], OP.add)
            nc.vector.tensor_scalar(mk[:], mk[:], 0.5, EPS, OP.mult, OP.add)
            rk = rms_r(mk, "rk")
            kn0 = wp.tile([128, 16], f32, tag="kn0")
            nc.vector.tensor_tensor(kn0[:], k0, rk[:], OP.mult)
            akm = wp.tile([128, 16], f32, tag="akm")
            nc.vector.tensor_tensor(akm[:], kn0[:], kn0[:], OP.mult)
            nc.vector.tensor_scalar(akm[:], akm[:], 0.5, EPS, OP.mult, OP.add)
            rak = rms_r(akm, "rak")
            ak = wp.tile([128, 16], f32, tag="ak")
            nc.vector.tensor_tensor(ak[:], kn0[:], rak[:], OP.mult)
            kxr = wp.tile([128, 16], f32, tag="kxr")
            nc.vector.tensor_tensor(kxr[:], ak[:], cosk, OP.mult)
            kyr = wp.tile([128, 16], f32, tag="kyr")
            nc.vector.tensor_tensor(kyr[:], ak[:], sink, OP.mult)

            Kop = cp.tile([3, 2048], f32, tag="Kop")
            nc.sync.dma_start(Kop[0:1, :], kxr[:])
            nc.sync.dma_start(Kop[1:2, :], kyr[:])
            ones16 = wp.tile([128, 16], f32, tag="ones16")
            nc.gpsimd.memset(ones16[:], 1.0)
            nc.sync.dma_start(Kop[2:3, :], ones16[:])

            # ---------- key pipeline block-major for v ([128,16]) --------
            b0, b1 = xkb[:, 0:16], xkb[:, 16:32]
            bsq = wp.tile([128, 16], f32, tag="bsq")
            nc.vector.tensor_tensor(bsq[:], b0, b0, OP.mult)
            bsq1 = wp.tile([128, 16], f32, tag="bsq1")
            nc.vector.tensor_tensor(bsq1[:], b1, b1, OP.mult)
            mb = wp.tile([128, 16], f32, tag="mb")
            nc.vector.tensor_tensor(mb[:], bsq[:], bsq1[:], OP.add)
            nc.vector.tensor_scalar(mb[:], mb[:], 0.5, EPS, OP.mult, OP.add)
            rb = rms_r(mb, "rb")
            vb = cp.tile([128, 32], f32, tag="vb")  # [v | ones]
            nc.gpsimd.memset(vb[:, 16:32], 1.0)
            xn1b = wp.tile([128, 16], f32, tag="xn1b")
            nc.vector.tensor_tensor(xn1b[:], b1, rb[:], OP.mult)
            nc.vector.tensor_scalar(vb[:, 0:16], xn1b[:], bc[:, 2:3], None,
                                    OP.mult)

            # ---------- main attention loop ----------
            ND = cp.tile([2, 4096], f32, tag="ND")
            for c in range(NQC):
                nd_ps = psnd.tile([2, 512], f32)
                nk = 2 * c + 2
                for k in range(nk):
                    ps = pslog.tile([128, 512], f32)
                    nc.tensor.matmul(ps[:], Kop[:, 128 * k:128 * (k + 1)],
                                     Qop[:, 512 * c:512 * (c + 1)],
                                     start=True, stop=True)
                    e = ep.tile([128, 512], f32)
                    nc.scalar.activation(e[:], ps[:], AF.Exp)
                    if k >= nk - 2:
                        m = k - (nk - 2)
                        nc.vector.tensor_tensor(
                            e[:], e[:], masks[:, 512 * m:512 * (m + 1)],
                            OP.mult)
                    nc.tensor.matmul(nd_ps[:], vb[:, k::16], e[:],
                                     start=(k == 0), stop=(k == nk - 1))
                nc.vector.tensor_copy(ND[:, 512 * c:512 * (c + 1)], nd_ps[:])

            # ---------- allreduce N/D with pair core ----------
            nd_in = dp.tile([2, 4096], f32)
            nd_out = dp.tile([2, 4096], f32)
            nc.sync.dma_start(nd_in[:], ND[:])
            nc.gpsimd.collective_compute(
                "AllReduce", OP.add,
                replica_groups=[[0, 1], [2, 3], [4, 5], [6, 7]],
                ins=[nd_in.opt()], outs=[nd_out.opt()])
            Nrm = wp.tile([128, 32], f32, tag="Nrm")
            nc.sync.dma_start(Nrm[:], nd_out[0:1, :])
            Drm = wp.tile([128, 32], f32, tag="Drm")
            nc.sync.dma_start(Drm[:], nd_out[1:2, :])

            # ---------- finalize: o0, residual, gated MLP ----------
            rD = wp.tile([128, 32], f32, tag="rD")
            nc.vector.reciprocal(rD[:], Drm[:])
            o0 = wp.tile([128, 32], f32, tag="o0")
            nc.vector.tensor_tensor(o0[:], Nrm[:], rD[:], OP.mult)
            outt = cp.tile([128, 64], f32, tag="outt")
            h1 = wp.tile([128, 32], f32, tag="h1")
            nc.vector.tensor_tensor(h1[:], x1, o0[:], OP.add)
            hsq0 = wp.tile([128, 32], f32, tag="hsq0")
            nc.vector.tensor_tensor(hsq0[:], x0, x0, OP.mult)
            hsq1 = wp.tile([128, 32], f32, tag="hsq1")
            nc.vector.tensor_tensor(hsq1[:], h1[:], h1[:], OP.mult)
            mh = wp.tile([128, 32], f32, tag="mh")
            nc.vector.tensor_tensor(mh[:], hsq0[:], hsq1[:], OP.add)
            nc.vector.tensor_scalar(mh[:], mh[:], 0.5, EPS, OP.mult, OP.add)
            rh = rms_r(mh, "rh")
            hn0 = wp.tile([128, 32], f32, tag="hn0")
            nc.vector.tensor_tensor(hn0[:], x0, rh[:], OP.mult)
            hn1 = wp.tile([128, 32], f32, tag="hn1")
            nc.vector.tensor_tensor(hn1[:], h1[:], rh[:], OP.mult)
            g0 = wp.tile([128, 32], f32, tag="g0")
            nc.vector.tensor_scalar(g0[:], hn0[:], bc[:, 3:4], None, OP.mult)
            gt = wp.tile([128, 32], f32, tag="gt")
            nc.vector.tensor_scalar(gt[:], hn1[:], bc[:, 4:5], None, OP.mult)
            nc.vector.tensor_tensor(g0[:], g0[:], gt[:], OP.add)
            g1 = wp.tile([128, 32], f32, tag="g1")
            nc.vector.tensor_scalar(g1[:], hn0[:], bc[:, 5:6], None, OP.mult)
            nc.vector.tensor_tensor(g1[:], g1[:], gt[:], OP.add)

            def silu(g, tag):
                eg = wp.tile([128, 32], f32, tag=tag + "_e")
                nc.scalar.activation(eg[:], g[:], AF.Exp, scale=-1.0)
                nc.vector.tensor_scalar(eg[:], eg[:], 1.0, None, OP.add)
                rg = wp.tile([128, 32], f32, tag=tag + "_r")
                nc.vector.reciprocal(rg[:], eg[:])
                sg = wp.tile([128, 32], f32, tag=tag + "_s")
                nc.vector.tensor_tensor(sg[:], g[:], rg[:], OP.mult)
                return sg

            s0 = silu(g0, "s0")
            s1 = silu(g1, "s1")
            df = wp.tile([128, 32], f32, tag="df")
            nc.vector.tensor_tensor(df[:], s1[:], s0[:], OP.subtract)
            nc.vector.tensor_tensor(df[:], df[:], hn0[:], OP.mult)
            nc.vector.tensor_scalar(df[:], df[:], bc[:, 6:7], None, OP.mult)
            nc.vector.tensor_copy(outt[:, 0:32], x0)
            nc.vector.tensor_tensor(outt[:, 32:64], h1[:], df[:], OP.add)
            nc.sync.dma_start(out_d[:], outt[:])

    nc.compile()
    return nc


def _host_inputs(x, mask, q_weight, v_weight, gate_weight, carry_weight):
    """Build the 8 per-core input maps. Host work is layout/indexing only."""
    f32 = np.float32
    x = np.ascontiguousarray(x, dtype=f32)
    theta = np.arange(L, dtype=f32) * f32(OMEGA)
    cth = np.cos(theta).astype(f32)
    sth = np.sin(theta).astype(f32)

    # query-side layouts (per batch): [128, 64] row-major pos = 32p+n
    def rm64(a0, a1):
        out = np.empty((128, 64), f32)
        out[:, 0:32] = a0.reshape(128, 32)
        out[:, 32:64] = a1.reshape(128, 32)
        return out

    tq = rm64(cth, sth)
    qidx = np.arange(L).reshape(128, 32)  # pos of (p, n)

    in_maps = []
    for core in range(N_CORES):
        b, par = core // 2, core % 2
        kb = 2 * np.arange(NKB) + par              # global key blocks
        keys = (kb[:, None] * 128 + np.arange(128)[None, :]).reshape(-1)
        xg = x[b][keys]                            # [2048, 2] gathered keys
        cg, sg = cth[keys], sth[keys]
        # row-major over gathered order t = 16p+n
        xkr = np.empty((128, 32), f32)
        xkr[:, 0:16] = xg[:, 0].reshape(128, 16)
        xkr[:, 16:32] = xg[:, 1].reshape(128, 16)
        tkr = np.empty((128, 32), f32)
        tkr[:, 0:16] = cg.reshape(128, 16)
        tkr[:, 16:32] = sg.reshape(128, 16)
        # block-major: xkb[p, k] = xg[128k+p]
        xkb = np.empty((128, 32), f32)
        xkb[:, 0:16] = xg[:, 0].reshape(16, 128).T
        xkb[:, 16:32] = xg[:, 1].reshape(16, 128).T
        # masks for the two diagonal-crossing tiles (k = 2c, 2c+1)
        kk = np.arange(128)[:, None]
        qq = np.arange(512)[None, :]
        m = np.empty((128, 1024), f32)
        m[:, 0:512] = (kk + 128 * par <= qq)
        m[:, 512:1024] = (kk + 128 * (2 + par) <= qq)
        in_maps.append({
            "xq": rm64(x[b, :, 0], x[b, :, 1]),
            "tq": tq,
            "xkr": xkr, "tkr": tkr, "xkb": xkb, "masks": m,
            "wq": np.asarray(q_weight, f32),
            "wv": np.asarray(v_weight, f32),
            "wg": np.asarray(gate_weight, f32),
            "wc": np.asarray(carry_weight, f32),
        })
    return in_maps


def kernel(x, mask, q_weight, v_weight, gate_weight, carry_weight,
           _want_results=False):
    global _compiled
    from concourse.bass_utils import run_bass_kernel_spmd

    if _compiled is None:
        _compiled = _build()
    in_maps = _host_inputs(x, mask, q_weight, v_weight, gate_weight,
                           carry_weight)
    res = run_bass_kernel_spmd(_compiled, in_maps, list(range(N_CORES)))
    out = np.empty((B, L, 2), np.float32)
    for b in range(B):
        r = res.results[2 * b]["out"]  # [128, 64]; pair core is identical
        out[b, :, 0] = r[:, 0:32].reshape(-1)
        out[b, :, 1] = r[:, 32:64].reshape(-1)
    if _want_results:
        return out, res
    return out



# revision 6
# speedup vs baseline: 1.9100x; 1.9100x over previous
import math
import sys

sys.path.insert(0, "/opt/trn_rl_repo")

import numpy as np

# ---- model constants (from the reference nn.Module) ----
ROPE_PERIOD = 19.0
OMEGA = 2.0 * math.pi / ROPE_PERIOD
PEAK_EPS = 0.3
TARGET_LOGIT_GAP = math.log(10.0)
ATTN_AMPLITUDE = TARGET_LOGIT_GAP / (
    math.cos(OMEGA * PEAK_EPS) - math.cos(OMEGA * (1.0 - PEAK_EPS))
)
QK_NORM_SCALE = math.sqrt(ATTN_AMPLITUDE / math.sqrt(2.0))
SCALE = 2.0 ** (-0.5) * QK_NORM_SCALE**2
EMBED_CONST = 1000.0
EPS = 1e-6

B, L = 4, 4096
N_CORES = 8
SQRT2 = math.sqrt(2.0)

# Query-chunk split across the core pair of each batch: both sets cost 36
# causal tile-pairs, so no cross-core N/D reduction is needed. The SPMD
# instruction stream pads each slot to P pairs; surplus tiles are zeroed
# by the host-staged mask blob.
CHUNKS = [[0, 3, 4, 7], [1, 2, 5, 6]]
PPAIRS = [4, 8, 12, 16]        # static pairs per slot (max over parities)
NMASK = 4                      # masked pairs at the tail of each slot

# product-block pattern for the triple-bf16-split logits matmul:
# logit = sum over (a,b) in {(0,0),(1,0),(0,1),(2,0),(1,1),(0,2)} of q_a.k_b
QLVL = [0, 1, 0, 2, 1, 0]
KLVL = [0, 0, 1, 0, 1, 2]

_compiled = None


def _build():
    import concourse.bass as bass
    import concourse.tile as tile
    from concourse import bacc, mybir

    f32 = mybir.dt.float32
    f32r = mybir.dt.float32r
    bf16 = mybir.dt.bfloat16
    AF = mybir.ActivationFunctionType
    OP = mybir.AluOpType

    nc = bacc.Bacc("TRN2", target_bir_lowering=False, debug=False,
                   num_devices=N_CORES)

    # per-core inputs (host pre-arranged layouts; pure indexing, no math)
    xq_d = nc.dram_tensor("xq", [128, 32], f32, kind="ExternalInput").ap()
    tq_d = nc.dram_tensor("tq", [128, 32], f32, kind="ExternalInput").ap()
    xk_d = nc.dram_tensor("xk", [128, 64], f32, kind="ExternalInput").ap()
    tk_d = nc.dram_tensor("tk", [128, 64], f32, kind="ExternalInput").ap()
    xkb_d = nc.dram_tensor("xkb", [128, 64], f32, kind="ExternalInput").ap()
    mask_d = nc.dram_tensor("masks", [128, 16 * 1024], mybir.dt.bfloat16,
                            kind="ExternalInput").ap()
    wq_d = nc.dram_tensor("wq", [1], f32, kind="ExternalInput").ap()
    wv_d = nc.dram_tensor("wv", [1], f32, kind="ExternalInput").ap()
    wg_d = nc.dram_tensor("wg", [2], f32, kind="ExternalInput").ap()
    wc_d = nc.dram_tensor("wc", [1], f32, kind="ExternalInput").ap()
    out_d = nc.dram_tensor("out", [128, 32], f32, kind="ExternalOutput").ap()

    with tile.TileContext(nc) as tc:
        with (
            tc.tile_pool(name="const", bufs=1) as cp,
            tc.tile_pool(name="work", bufs=2) as wp,
            tc.tile_pool(name="ep", bufs=3) as ep,
            tc.tile_pool(name="pslog", bufs=2, space="PSUM") as pslog,
            tc.tile_pool(name="psnd", bufs=2, space="PSUM") as psnd,
            tc.tile_pool(name="psbc", bufs=1, space="PSUM") as psbc,
        ):
            # ---------- load inputs ----------
            xq = cp.tile([128, 32], f32, tag="xq")
            nc.sync.dma_start(xq[:], xq_d[:])
            tq = cp.tile([128, 32], f32, tag="tq")
            nc.sync.dma_start(tq[:], tq_d[:])
            xk = cp.tile([128, 64], f32, tag="xk")
            nc.sync.dma_start(xk[:], xk_d[:])
            tk = cp.tile([128, 64], f32, tag="tk")
            nc.sync.dma_start(tk[:], tk_d[:])
            xkb = cp.tile([128, 64], f32, tag="xkb")
            nc.sync.dma_start(xkb[:], xkb_d[:])
            masks = cp.tile([128, 16 * 1024], bf16, tag="masks")
            nc.sync.dma_start(masks[:], mask_d[:])

            sw = cp.tile([1, 8], f32, tag="sw")  # scalar workspace row
            nc.sync.dma_start(sw[0:1, 0:1], wq_d[0:1])
            nc.sync.dma_start(sw[0:1, 1:2], wv_d[0:1])
            nc.sync.dma_start(sw[0:1, 2:4], wg_d[0:2])
            nc.sync.dma_start(sw[0:1, 4:5], wc_d[0:1])

            # ---------- scalar prep: cos/sin(phi) by Taylor, gate consts ----
            # layout of wvec [1, 8]: 0=cS 1=sS 2=wv 3=ga 4=gc 5=ga2 6=wc
            wvec = cp.tile([1, 8], f32, tag="wvec")
            t2 = cp.tile([1, 1], f32, tag="t2")
            nc.scalar.activation(t2[:], sw[0:1, 0:1], AF.Square)
            u = cp.tile([1, 1], f32, tag="u")
            nc.vector.tensor_scalar(u[:], t2[:], -1.0 / 720.0, 1.0 / 24.0,
                                    OP.mult, OP.add)
            nc.vector.tensor_scalar(u[:], u[:], t2[0:1, 0:1], -0.5,
                                    OP.mult, OP.add)
            nc.vector.tensor_scalar(u[:], u[:], t2[0:1, 0:1], 1.0,
                                    OP.mult, OP.add)
            nc.vector.tensor_scalar(wvec[0:1, 0:1], u[:], SCALE, None, OP.mult)
            v = cp.tile([1, 1], f32, tag="v")
            nc.vector.tensor_scalar(v[:], t2[:], -1.0 / 5040.0, 1.0 / 120.0,
                                    OP.mult, OP.add)
            nc.vector.tensor_scalar(v[:], v[:], t2[0:1, 0:1], -1.0 / 6.0,
                                    OP.mult, OP.add)
            nc.vector.tensor_scalar(v[:], v[:], t2[0:1, 0:1], 1.0,
                                    OP.mult, OP.add)
            nc.vector.tensor_scalar(v[:], v[:], sw[0:1, 0:1], None, OP.mult)
            nc.vector.tensor_scalar(wvec[0:1, 1:2], v[:], SCALE, None, OP.mult)
            nc.vector.tensor_copy(wvec[0:1, 2:3], sw[0:1, 1:2])
            nc.vector.tensor_copy(wvec[0:1, 3:4], sw[0:1, 2:3])
            nc.vector.tensor_copy(wvec[0:1, 4:5], sw[0:1, 3:4])
            nc.vector.tensor_scalar(wvec[0:1, 5:6], sw[0:1, 3:4],
                                    -1.0 / EMBED_CONST, sw[0:1, 2:3],
                                    OP.mult, OP.add)
            nc.vector.tensor_copy(wvec[0:1, 6:7], sw[0:1, 4:5])
            nc.vector.tensor_scalar(wvec[0:1, 7:8], sw[0:1, 4:5], 0.0, None,
                                    OP.mult)

            # broadcast scalars to all 128 partitions via K=1 matmul
            ones_row = cp.tile([1, 128], f32, tag="ones_row")
            nc.gpsimd.memset(ones_row[:], 1.0)
            bc_ps = psbc.tile([128, 8], f32)
            nc.tensor.matmul(bc_ps[:], ones_row[:], wvec[:], start=True,
                             stop=True)
            bc = cp.tile([128, 8], f32, tag="bc")
            nc.vector.tensor_copy(bc[:], bc_ps[:])

            def rms_r(x2sum, tag):
                # x2sum = sum of squares*0.5+eps -> returns rsqrt tile
                ln = wp.tile(x2sum.shape, f32, tag=tag + "_ln")
                nc.scalar.activation(ln[:], x2sum[:], AF.Ln)
                r = wp.tile(x2sum.shape, f32, tag=tag + "_r")
                nc.scalar.activation(r[:], ln[:], AF.Exp, scale=-0.5)
                return r

            def split3(src, w, tag):
                # triple bf16 split: returns [lvl0, lvl1, lvl2] bf16 tiles
                l0 = cp.tile([128, w], bf16, tag=tag + "0")
                nc.vector.tensor_copy(l0[:], src[:])
                r1 = wp.tile([128, w], f32, tag=tag + "r1")
                nc.vector.tensor_tensor(r1[:], src[:], l0[:], OP.subtract)
                l1 = cp.tile([128, w], bf16, tag=tag + "1")
                nc.vector.tensor_copy(l1[:], r1[:])
                r2 = wp.tile([128, w], f32, tag=tag + "r2")
                nc.vector.tensor_tensor(r2[:], r1[:], l1[:], OP.subtract)
                l2 = cp.tile([128, w], bf16, tag=tag + "2")
                nc.vector.tensor_copy(l2[:], r2[:])
                return [l0, l1, l2]

            # ---------- query pipeline ([128,16], local pos = 16p+m) -------
            x0q, x1q = xq[:, 0:16], xq[:, 16:32]
            cosq, sinq = tq[:, 0:16], tq[:, 16:32]
            sq = wp.tile([128, 16], f32, tag="sq")
            nc.vector.tensor_tensor(sq[:], x0q, x0q, OP.mult)
            sq1 = wp.tile([128, 16], f32, tag="sq1")
            nc.vector.tensor_tensor(sq1[:], x1q, x1q, OP.mult)
            mq = wp.tile([128, 16], f32, tag="mq")
            nc.vector.tensor_tensor(mq[:], sq[:], sq1[:], OP.add)
            nc.vector.tensor_scalar(mq[:], mq[:], 0.5, EPS, OP.mult, OP.add)
            rq = rms_r(mq, "rq")
            xn0 = wp.tile([128, 16], f32, tag="xn0")
            nc.vector.tensor_tensor(xn0[:], x0q, rq[:], OP.mult)
            am = wp.tile([128, 16], f32, tag="am")
            nc.vector.tensor_tensor(am[:], xn0[:], xn0[:], OP.mult)
            nc.vector.tensor_scalar(am[:], am[:], 0.5, EPS, OP.mult, OP.add)
            ra = rms_r(am, "ra")
            aq = wp.tile([128, 16], f32, tag="aq")
            nc.vector.tensor_tensor(aq[:], xn0[:], ra[:], OP.mult)
            qz = wp.tile([128, 16], f32, tag="qzt")
            nc.scalar.activation(qz[:], aq[:], AF.Abs)
            nc.vector.tensor_scalar(qz[:], qz[:], -SCALE * SQRT2, None, OP.mult)
            t1 = wp.tile([128, 16], f32, tag="t1")
            nc.vector.tensor_scalar(t1[:], cosq, bc[:, 0:1], None, OP.mult)
            t2q = wp.tile([128, 16], f32, tag="t2q")
            nc.vector.tensor_scalar(t2q[:], sinq, bc[:, 1:2], None, OP.mult)
            nc.vector.tensor_tensor(t1[:], t1[:], t2q[:], OP.add)
            qx = wp.tile([128, 16], f32, tag="qx")
            nc.vector.tensor_tensor(qx[:], aq[:], t1[:], OP.mult)
            t3 = wp.tile([128, 16], f32, tag="t3")
            nc.vector.tensor_scalar(t3[:], sinq, bc[:, 0:1], None, OP.mult)
            t4 = wp.tile([128, 16], f32, tag="t4")
            nc.vector.tensor_scalar(t4[:], cosq, bc[:, 1:2], None, OP.mult)
            nc.vector.tensor_tensor(t3[:], t3[:], t4[:], OP.subtract)
            qy = wp.tile([128, 16], f32, tag="qy")
            nc.vector.tensor_tensor(qy[:], aq[:], t3[:], OP.mult)

            qxs = split3(qx, 16, "qx")
            qys = split3(qy, 16, "qy")
            qzs = split3(qz, 16, "qz")

            Qop = cp.tile([18, 2048], bf16, tag="Qop")
            for i, a in enumerate(QLVL):
                nc.sync.dma_start(Qop[3 * i:3 * i + 1, :], qxs[a][:])
                nc.sync.dma_start(Qop[3 * i + 1:3 * i + 2, :], qys[a][:])
                nc.sync.dma_start(Qop[3 * i + 2:3 * i + 3, :], qzs[a][:])

            # ---------- key pipeline ([128,32], t = 32p+n) ------------------
            k0, k1 = xk[:, 0:32], xk[:, 32:64]
            cosk, sink = tk[:, 0:32], tk[:, 32:64]
            ksq = wp.tile([128, 32], f32, tag="ksq")
            nc.vector.tensor_tensor(ksq[:], k0, k0, OP.mult)
            ksq1 = wp.tile([128, 32], f32, tag="ksq1")
            nc.vector.tensor_tensor(ksq1[:], k1, k1, OP.mult)
            mk = wp.tile([128, 32], f32, tag="mk")
            nc.vector.tensor_tensor(mk[:], ksq[:], ksq1[:], OP.add)
            nc.vector.tensor_scalar(mk[:], mk[:], 0.5, EPS, OP.mult, OP.add)
            rk = rms_r(mk, "rk")
            kn0 = wp.tile([128, 32], f32, tag="kn0")
            nc.vector.tensor_tensor(kn0[:], k0, rk[:], OP.mult)
            akm = wp.tile([128, 32], f32, tag="akm")
            nc.vector.tensor_tensor(akm[:], kn0[:], kn0[:], OP.mult)
            nc.vector.tensor_scalar(akm[:], akm[:], 0.5, EPS, OP.mult, OP.add)
            rak = rms_r(akm, "rak")
            ak = wp.tile([128, 32], f32, tag="ak")
            nc.vector.tensor_tensor(ak[:], kn0[:], rak[:], OP.mult)
            kx = wp.tile([128, 32], f32, tag="kx")
            nc.vector.tensor_tensor(kx[:], ak[:], cosk, OP.mult)
            ky = wp.tile([128, 32], f32, tag="ky")
            nc.vector.tensor_tensor(ky[:], ak[:], sink, OP.mult)

            kxs = split3(kx, 32, "kxl")
            kys = split3(ky, 32, "kyl")

            ones32 = cp.tile([128, 32], bf16, tag="ones32")
            nc.gpsimd.memset(ones32[:], 1.0)
            zeros32 = cp.tile([128, 32], bf16, tag="zeros32")
            nc.gpsimd.memset(zeros32[:], 0.0)
            Kop = cp.tile([18, 4096], bf16, tag="Kop")
            for i, bl in enumerate(KLVL):
                nc.sync.dma_start(Kop[3 * i:3 * i + 1, :], kxs[bl][:])
                nc.sync.dma_start(Kop[3 * i + 1:3 * i + 2, :], kys[bl][:])
                nc.sync.dma_start(Kop[3 * i + 2:3 * i + 3, :],
                                  ones32[:] if bl == 0 else zeros32[:])

            # ---------- v pipeline block-major ([128,32], block g) ----------
            vb0, vb1 = xkb[:, 0:32], xkb[:, 32:64]
            bsq = wp.tile([128, 32], f32, tag="bsq")
            nc.vector.tensor_tensor(bsq[:], vb0, vb0, OP.mult)
            bsq1 = wp.tile([128, 32], f32, tag="bsq1")
            nc.vector.tensor_tensor(bsq1[:], vb1, vb1, OP.mult)
            mb = wp.tile([128, 32], f32, tag="mb")
            nc.vector.tensor_tensor(mb[:], bsq[:], bsq1[:], OP.add)
            nc.vector.tensor_scalar(mb[:], mb[:], 0.5, EPS, OP.mult, OP.add)
            rb = rms_r(mb, "rb")
            xn1b = wp.tile([128, 32], f32, tag="xn1b")
            nc.vector.tensor_tensor(xn1b[:], vb1, rb[:], OP.mult)
            vv = wp.tile([128, 32], f32, tag="vv")
            nc.vector.tensor_scalar(vv[:], xn1b[:], bc[:, 2:3], None, OP.mult)
            # vbt cols per block g: [vh | vl | ones] at 3g..3g+2, all f32r
            vbt = cp.tile([128, 96], f32r, tag="vbt")
            nc.vector.tensor_copy(vbt[:, 0:96:3], vv[:])
            vlo = wp.tile([128, 32], f32, tag="vlo")
            nc.vector.tensor_tensor(vlo[:], vv[:], vbt[:, 0:96:3], OP.subtract)
            nc.vector.tensor_copy(vbt[:, 1:96:3], vlo[:])
            onesf = cp.tile([128, 32], f32, tag="onesf")
            nc.gpsimd.memset(onesf[:], 1.0)
            nc.vector.tensor_copy(vbt[:, 2:96:3], onesf[:])

            # ---------- main attention loop ----------
            # NDall: per chunk slot s, [3,512] N/D psum rows land in
            # partitions 32s..32s+31 as 16-wide col groups Nh|Nl|D.
            NDall = cp.tile([128, 48], f32, tag="NDall")
            for s in range(4):
                P = PPAIRS[s]
                nd_ps = psnd.tile([3, 512], f32)
                for j in range(P):
                    ps = pslog.tile([128, 1024], f32)
                    nc.tensor.matmul(ps[:, 0:512],
                                     Kop[:, 256 * j:256 * j + 128],
                                     Qop[:, 512 * s:512 * (s + 1)],
                                     start=True, stop=True)
                    nc.tensor.matmul(ps[:, 512:1024],
                                     Kop[:, 256 * j + 128:256 * j + 256],
                                     Qop[:, 512 * s:512 * (s + 1)],
                                     start=True, stop=True)
                    e = ep.tile([128, 1024], f32r)
                    nc.scalar.activation(e[:], ps[:], AF.Exp)
                    if j >= P - NMASK:
                        m = 4 * s + (j - (P - NMASK))
                        nc.vector.tensor_tensor(
                            e[:], e[:], masks[:, 1024 * m:1024 * (m + 1)],
                            OP.mult)
                    nc.tensor.matmul(nd_ps[:], vbt[:, 6 * j:6 * j + 3],
                                     e[:, 0:512],
                                     start=(j == 0), stop=False)
                    nc.tensor.matmul(nd_ps[:], vbt[:, 6 * j + 3:6 * j + 6],
                                     e[:, 512:1024],
                                     start=False, stop=(j == P - 1))
                nds = wp.tile([3, 512], f32, tag="nds")
                nc.vector.tensor_copy(nds[:], nd_ps[:])
                nc.sync.dma_start(NDall[32 * s:32 * s + 32, 0:16],
                                  nds[0:1, :])
                nc.sync.dma_start(NDall[32 * s:32 * s + 32, 16:32],
                                  nds[1:2, :])
                nc.sync.dma_start(NDall[32 * s:32 * s + 32, 32:48],
                                  nds[2:3, :])

            # ---------- finalize: o0, residual, gated MLP ([128,16]) -------
            Nrm = wp.tile([128, 16], f32, tag="Nrm")
            nc.vector.tensor_tensor(Nrm[:], NDall[:, 0:16], NDall[:, 16:32],
                                    OP.add)
            rD = wp.tile([128, 16], f32, tag="rD")
            nc.vector.reciprocal(rD[:], NDall[:, 32:48])
            o0 = wp.tile([128, 16], f32, tag="o0")
            nc.vector.tensor_tensor(o0[:], Nrm[:], rD[:], OP.mult)
            outt = cp.tile([128, 32], f32, tag="outt")
            h1 = wp.tile([128, 16], f32, tag="h1")
            nc.vector.tensor_tensor(h1[:], x1q, o0[:], OP.add)
            hsq0 = wp.tile([128, 16], f32, tag="hsq0")
            nc.vector.tensor_tensor(hsq0[:], x0q, x0q, OP.mult)
            hsq1 = wp.tile([128, 16], f32, tag="hsq1")
            nc.vector.tensor_tensor(hsq1[:], h1[:], h1[:], OP.mult)
            mh = wp.tile([128, 16], f32, tag="mh")
            nc.vector.tensor_tensor(mh[:], hsq0[:], hsq1[:], OP.add)
            nc.vector.tensor_scalar(mh[:], mh[:], 0.5, EPS, OP.mult, OP.add)
            rh = rms_r(mh, "rh")
            hn0 = wp.tile([128, 16], f32, tag="hn0")
            nc.vector.tensor_tensor(hn0[:], x0q, rh[:], OP.mult)
            hn1 = wp.tile([128, 16], f32, tag="hn1")
            nc.vector.tensor_tensor(hn1[:], h1[:], rh[:], OP.mult)
            g0 = wp.tile([128, 16], f32, tag="g0")
            nc.vector.tensor_scalar(g0[:], hn0[:], bc[:, 3:4], None, OP.mult)
            gt = wp.tile([128, 16], f32, tag="gt")
            nc.vector.tensor_scalar(gt[:], hn1[:], bc[:, 4:5], None, OP.mult)
            nc.vector.tensor_tensor(g0[:], g0[:], gt[:], OP.add)
            g1 = wp.tile([128, 16], f32, tag="g1")
            nc.vector.tensor_scalar(g1[:], hn0[:], bc[:, 5:6], None, OP.mult)
            nc.vector.tensor_tensor(g1[:], g1[:], gt[:], OP.add)

            def silu(g, tag):
                eg = wp.tile([128, 16], f32, tag=tag + "_e")
                nc.scalar.activation(eg[:], g[:], AF.Exp, scale=-1.0)
                nc.vector.tensor_scalar(eg[:], eg[:], 1.0, None, OP.add)
                rg = wp.tile([128, 16], f32, tag=tag + "_r")
                nc.vector.reciprocal(rg[:], eg[:])
                sg = wp.tile([128, 16], f32, tag=tag + "_s")
                nc.vector.tensor_tensor(sg[:], g[:], rg[:], OP.mult)
                return sg

            s0 = silu(g0, "s0")
            s1 = silu(g1, "s1")
            df = wp.tile([128, 16], f32, tag="df")
            nc.vector.tensor_tensor(df[:], s1[:], s0[:], OP.subtract)
            nc.vector.tensor_tensor(df[:], df[:], hn0[:], OP.mult)
            nc.vector.tensor_scalar(df[:], df[:], bc[:, 6:7], None, OP.mult)
            nc.vector.tensor_copy(outt[:, 0:16], x0q)
            nc.vector.tensor_tensor(outt[:, 16:32], h1[:], df[:], OP.add)
            nc.sync.dma_start(out_d[:], outt[:])

    nc.compile()
    return nc


def _host_inputs(x, mask, q_weight, v_weight, gate_weight, carry_weight):
    """Build the 8 per-core input maps. Host work is layout/indexing only."""
    f32 = np.float32
    x = np.ascontiguousarray(x, dtype=f32)
    theta = np.arange(L, dtype=f32) * f32(OMEGA)
    cth = np.cos(theta).astype(f32)
    sth = np.sin(theta).astype(f32)

    kk = np.arange(128)[:, None]
    qq = np.arange(512)[None, :]

    # key-side layouts shared by the two cores of a batch
    def rm(a0, a1, w):
        out = np.empty((128, 2 * w), f32)
        out[:, 0:w] = a0.reshape(128, w)
        out[:, w:2 * w] = a1.reshape(128, w)
        return out

    tk = rm(cth, sth, 32)

    in_maps = []
    for core in range(N_CORES):
        b, h = core // 2, core % 2
        chunks = CHUNKS[h]
        # query-side: local pos = 16p+m over 4 slots of 512
        qpos = np.concatenate([np.arange(512) + 512 * C for C in chunks])
        xq = rm(x[b, qpos, 0], x[b, qpos, 1], 16)
        tq = rm(cth[qpos], sth[qpos], 16)
        # key-side row-major t = 32p+n
        xk = rm(x[b, :, 0], x[b, :, 1], 32)
        # block-major: xkb[p, g] = x[128g+p]
        xkb = np.empty((128, 64), f32)
        xkb[:, 0:32] = x[b, :, 0].reshape(32, 128).T
        xkb[:, 32:64] = x[b, :, 1].reshape(32, 128).T
        # mask blob: per slot s, the last NMASK static pairs are masked.
        m = np.empty((128, 16 * 1024), np.float32)
        for s, C in enumerate(chunks):
            P = PPAIRS[s]
            for j4 in range(NMASK):
                j = P - NMASK + j4
                for side in range(2):
                    g = 2 * j + side
                    col = (4 * s + j4) * 1024 + side * 512
                    m[:, col:col + 512] = (128 * g + kk <= 512 * C + qq)
        in_maps.append({
            "xq": xq, "tq": tq, "xk": xk, "tk": tk, "xkb": xkb,
            "masks": m.astype(np.dtype("bfloat16") if False else f32),
            "wq": np.asarray(q_weight, f32),
            "wv": np.asarray(v_weight, f32),
            "wg": np.asarray(gate_weight, f32),
            "wc": np.asarray(carry_weight, f32),
        })
    # convert masks to bf16 via ml_dtypes
    import ml_dtypes
    for im in in_maps:
        im["masks"] = im["masks"].astype(ml_dtypes.bfloat16)
    return in_maps


def kernel(x, mask, q_weight, v_weight, gate_weight, carry_weight,
           _want_results=False):
    global _compiled
    from concourse.bass_utils import run_bass_kernel_spmd

    if _compiled is None:
        _compiled = _build()
    in_maps = _host_inputs(x, mask, q_weight, v_weight, gate_weight,
                           carry_weight)
    res = run_bass_kernel_spmd(_compiled, in_maps, list(range(N_CORES)))
    out = np.empty((B, L, 2), np.float32)
    for b in range(B):
        for h in range(2):
            r = res.results[2 * b + h]["out"]  # [128, 32]
            ch0 = r[:, 0:16].reshape(-1)
            ch1 = r[:, 16:32].reshape(-1)
            for s, C in enumerate(CHUNKS[h]):
                out[b, 512 * C:512 * (C + 1), 0] = ch0[512 * s:512 * (s + 1)]
                out[b, 512 * C:512 * (C + 1), 1] = ch1[512 * s:512 * (s + 1)]
    if _want_results:
        return out, res
    return out


# revision 9
# speedup vs baseline: 2.0875x; 1.0930x over previous
import math
import sys

sys.path.insert(0, "/opt/trn_rl_repo")

import numpy as np

# ---- model constants (from the reference nn.Module) ----
ROPE_PERIOD = 19.0
OMEGA = 2.0 * math.pi / ROPE_PERIOD
PEAK_EPS = 0.3
TARGET_LOGIT_GAP = math.log(10.0)
ATTN_AMPLITUDE = TARGET_LOGIT_GAP / (
    math.cos(OMEGA * PEAK_EPS) - math.cos(OMEGA * (1.0 - PEAK_EPS))
)
QK_NORM_SCALE = math.sqrt(ATTN_AMPLITUDE / math.sqrt(2.0))
SCALE = 2.0 ** (-0.5) * QK_NORM_SCALE**2
EMBED_CONST = 1000.0
EPS = 1e-6

B, L = 4, 4096
N_CORES = 8
SQRT2 = math.sqrt(2.0)

# Query-chunk split across the core pair of each batch: both sets cost 36
# causal tile-pairs, so no cross-core N/D reduction is needed. The SPMD
# instruction stream pads each slot to P pairs; surplus tiles are zeroed
# by the host-staged mask blob.
CHUNKS = [[0, 3, 4, 7], [1, 2, 5, 6]]
PPAIRS = [4, 8, 12, 16]        # static pairs per slot (max over parities)
NMASK = 4                      # masked pairs at the tail of each slot

# product-block pattern for the triple-bf16-split logits matmul:
# logit = sum over (a,b) in {(0,0),(1,0),(0,1),(2,0),(1,1),(0,2)} of q_a.k_b
QLVL = [0, 1, 0, 2, 1, 0]
KLVL = [0, 0, 1, 0, 1, 2]

_compiled = None


def _build():
    import concourse.bass as bass
    import concourse.tile as tile
    from concourse import bacc, mybir

    f32 = mybir.dt.float32
    f32r = mybir.dt.float32r
    bf16 = mybir.dt.bfloat16
    AF = mybir.ActivationFunctionType
    OP = mybir.AluOpType

    nc = bacc.Bacc("TRN2", target_bir_lowering=False, debug=False,
                   num_devices=N_CORES)

    # per-core inputs (host pre-arranged layouts; pure indexing, no math)
    xq_d = nc.dram_tensor("xq", [128, 32], f32, kind="ExternalInput").ap()
    tq_d = nc.dram_tensor("tq", [128, 32], f32, kind="ExternalInput").ap()
    xk_d = nc.dram_tensor("xk", [128, 64], f32, kind="ExternalInput").ap()
    tk_d = nc.dram_tensor("tk", [128, 64], f32, kind="ExternalInput").ap()
    xkb_d = nc.dram_tensor("xkb", [128, 64], f32, kind="ExternalInput").ap()
    mask_d = nc.dram_tensor("masks", [128, 16 * 1024], mybir.dt.bfloat16,
                            kind="ExternalInput").ap()
    wq_d = nc.dram_tensor("wq", [1], f32, kind="ExternalInput").ap()
    wv_d = nc.dram_tensor("wv", [1], f32, kind="ExternalInput").ap()
    wg_d = nc.dram_tensor("wg", [2], f32, kind="ExternalInput").ap()
    wc_d = nc.dram_tensor("wc", [1], f32, kind="ExternalInput").ap()
    out_d = nc.dram_tensor("out", [128, 32], f32, kind="ExternalOutput").ap()

    with tile.TileContext(nc) as tc:
        with (
            tc.tile_pool(name="const", bufs=1) as cp,
            tc.tile_pool(name="work", bufs=2) as wp,
            tc.tile_pool(name="ep", bufs=3) as ep,
            tc.tile_pool(name="pslog", bufs=3, space="PSUM") as pslog,
            tc.tile_pool(name="psnd", bufs=1, space="PSUM") as psnd,
            tc.tile_pool(name="psbc", bufs=1, space="PSUM") as psbc,
        ):
            # ---------- load inputs ----------
            xq = cp.tile([128, 32], f32, tag="xq")
            nc.sync.dma_start(xq[:], xq_d[:])
            tq = cp.tile([128, 32], f32, tag="tq")
            nc.sync.dma_start(tq[:], tq_d[:])
            xk = cp.tile([128, 64], f32, tag="xk")
            nc.sync.dma_start(xk[:], xk_d[:])
            tk = cp.tile([128, 64], f32, tag="tk")
            nc.sync.dma_start(tk[:], tk_d[:])
            xkb = cp.tile([128, 64], f32, tag="xkb")
            nc.sync.dma_start(xkb[:], xkb_d[:])
            masks = cp.tile([128, 16 * 1024], bf16, tag="masks")
            nc.sync.dma_start(masks[:], mask_d[:])

            sw = cp.tile([1, 8], f32, tag="sw")  # scalar workspace row
            nc.sync.dma_start(sw[0:1, 0:1], wq_d[0:1])
            nc.sync.dma_start(sw[0:1, 1:2], wv_d[0:1])
            nc.sync.dma_start(sw[0:1, 2:4], wg_d[0:2])
            nc.sync.dma_start(sw[0:1, 4:5], wc_d[0:1])

            # ---------- scalar prep: cos/sin(phi) by Taylor, gate consts ----
            # layout of wvec [1, 8]: 0=cS 1=sS 2=wv 3=ga 4=gc 5=ga2 6=wc
            wvec = cp.tile([1, 8], f32, tag="wvec")
            t2 = cp.tile([1, 1], f32, tag="t2")
            nc.vector.tensor_tensor(t2[:], sw[0:1, 0:1], sw[0:1, 0:1], OP.mult)
            u = cp.tile([1, 1], f32, tag="u")
            nc.vector.tensor_scalar(u[:], t2[:], -1.0 / 720.0, 1.0 / 24.0,
                                    OP.mult, OP.add)
            nc.vector.tensor_scalar(u[:], u[:], t2[0:1, 0:1], -0.5,
                                    OP.mult, OP.add)
            nc.vector.tensor_scalar(u[:], u[:], t2[0:1, 0:1], 1.0,
                                    OP.mult, OP.add)
            nc.vector.tensor_scalar(wvec[0:1, 0:1], u[:], SCALE, None, OP.mult)
            v = cp.tile([1, 1], f32, tag="v")
            nc.vector.tensor_scalar(v[:], t2[:], -1.0 / 5040.0, 1.0 / 120.0,
                                    OP.mult, OP.add)
            nc.vector.tensor_scalar(v[:], v[:], t2[0:1, 0:1], -1.0 / 6.0,
                                    OP.mult, OP.add)
            nc.vector.tensor_scalar(v[:], v[:], t2[0:1, 0:1], 1.0,
                                    OP.mult, OP.add)
            nc.vector.tensor_scalar(v[:], v[:], sw[0:1, 0:1], None, OP.mult)
            nc.vector.tensor_scalar(wvec[0:1, 1:2], v[:], SCALE, None, OP.mult)
            nc.vector.tensor_copy(wvec[0:1, 2:3], sw[0:1, 1:2])
            nc.vector.tensor_copy(wvec[0:1, 3:4], sw[0:1, 2:3])
            nc.vector.tensor_copy(wvec[0:1, 4:5], sw[0:1, 3:4])
            nc.vector.tensor_scalar(wvec[0:1, 5:6], sw[0:1, 3:4],
                                    -1.0 / EMBED_CONST, sw[0:1, 2:3],
                                    OP.mult, OP.add)
            nc.vector.tensor_copy(wvec[0:1, 6:7], sw[0:1, 4:5])
            nc.vector.tensor_scalar(wvec[0:1, 7:8], sw[0:1, 4:5], 0.0, None,
                                    OP.mult)

            # broadcast scalars to all 128 partitions via K=1 matmul
            ones_row = cp.tile([1, 128], f32, tag="ones_row")
            nc.gpsimd.memset(ones_row[:], 1.0)
            bc_ps = psbc.tile([128, 8], f32)
            nc.tensor.matmul(bc_ps[:], ones_row[:], wvec[:], start=True,
                             stop=True)
            bc = cp.tile([128, 8], f32, tag="bc")
            nc.vector.tensor_copy(bc[:], bc_ps[:])

            def rms_r(x2sum, tag):
                # x2sum = sum of squares*0.5+eps -> returns rsqrt tile
                ln = wp.tile(x2sum.shape, f32, tag=tag + "_ln")
                nc.scalar.activation(ln[:], x2sum[:], AF.Ln)
                r = wp.tile(x2sum.shape, f32, tag=tag + "_r")
                nc.scalar.activation(r[:], ln[:], AF.Exp, scale=-0.5)
                return r

            def split3(src, w, tag):
                # triple bf16 split: returns [lvl0, lvl1, lvl2] bf16 tiles
                l0 = cp.tile([128, w], bf16, tag=tag + "0")
                nc.vector.tensor_copy(l0[:], src[:])
                r1 = wp.tile([128, w], f32, tag=tag + "r1")
                nc.vector.tensor_tensor(r1[:], src[:], l0[:], OP.subtract)
                l1 = cp.tile([128, w], bf16, tag=tag + "1")
                nc.vector.tensor_copy(l1[:], r1[:])
                r2 = wp.tile([128, w], f32, tag=tag + "r2")
                nc.vector.tensor_tensor(r2[:], r1[:], l1[:], OP.subtract)
                l2 = cp.tile([128, w], bf16, tag=tag + "2")
                nc.vector.tensor_copy(l2[:], r2[:])
                return [l0, l1, l2]

            # ---------- query pipeline ([128,16], local pos = 16p+m) -------
            x0q, x1q = xq[:, 0:16], xq[:, 16:32]
            cosq, sinq = tq[:, 0:16], tq[:, 16:32]
            sq = wp.tile([128, 16], f32, tag="sq")
            nc.vector.tensor_tensor(sq[:], x0q, x0q, OP.mult)
            sq1 = wp.tile([128, 16], f32, tag="sq1")
            nc.vector.tensor_tensor(sq1[:], x1q, x1q, OP.mult)
            mq = wp.tile([128, 16], f32, tag="mq")
            nc.vector.tensor_tensor(mq[:], sq[:], sq1[:], OP.add)
            nc.vector.tensor_scalar(mq[:], mq[:], 0.5, EPS, OP.mult, OP.add)
            rq = rms_r(mq, "rq")
            xn0 = wp.tile([128, 16], f32, tag="xn0")
            nc.vector.tensor_tensor(xn0[:], x0q, rq[:], OP.mult)
            am = wp.tile([128, 16], f32, tag="am")
            nc.vector.tensor_tensor(am[:], xn0[:], xn0[:], OP.mult)
            nc.vector.tensor_scalar(am[:], am[:], 0.5, EPS, OP.mult, OP.add)
            ra = rms_r(am, "ra")
            aq = wp.tile([128, 16], f32, tag="aq")
            nc.vector.tensor_tensor(aq[:], xn0[:], ra[:], OP.mult)
            qz = wp.tile([128, 16], f32, tag="qzt")
            nc.vector.tensor_scalar(qz[:], aq[:], -1.0, None, OP.mult)
            nc.vector.tensor_tensor(qz[:], qz[:], aq[:], OP.max)
            nc.vector.tensor_scalar(qz[:], qz[:], -SCALE * SQRT2, None, OP.mult)
            t1 = wp.tile([128, 16], f32, tag="t1")
            nc.vector.tensor_scalar(t1[:], cosq, bc[:, 0:1], None, OP.mult)
            t2q = wp.tile([128, 16], f32, tag="t2q")
            nc.vector.tensor_scalar(t2q[:], sinq, bc[:, 1:2], None, OP.mult)
            nc.vector.tensor_tensor(t1[:], t1[:], t2q[:], OP.add)
            qx = wp.tile([128, 16], f32, tag="qx")
            nc.vector.tensor_tensor(qx[:], aq[:], t1[:], OP.mult)
            t3 = wp.tile([128, 16], f32, tag="t3")
            nc.vector.tensor_scalar(t3[:], sinq, bc[:, 0:1], None, OP.mult)
            t4 = wp.tile([128, 16], f32, tag="t4")
            nc.vector.tensor_scalar(t4[:], cosq, bc[:, 1:2], None, OP.mult)
            nc.vector.tensor_tensor(t3[:], t3[:], t4[:], OP.subtract)
            qy = wp.tile([128, 16], f32, tag="qy")
            nc.vector.tensor_tensor(qy[:], aq[:], t3[:], OP.mult)

            qxs = split3(qx, 16, "qx")
            qys = split3(qy, 16, "qy")
            qzs = split3(qz, 16, "qz")

            Qop = cp.tile([18, 2048], bf16, tag="Qop")
            for i, a in enumerate(QLVL):
                nc.sync.dma_start(Qop[3 * i:3 * i + 1, :], qxs[a][:])
                nc.sync.dma_start(Qop[3 * i + 1:3 * i + 2, :], qys[a][:])
                nc.sync.dma_start(Qop[3 * i + 2:3 * i + 3, :], qzs[a][:])

            # ---------- key pipeline ([128,32], t = 32p+n) ------------------
            k0, k1 = xk[:, 0:32], xk[:, 32:64]
            cosk, sink = tk[:, 0:32], tk[:, 32:64]
            ksq = wp.tile([128, 32], f32, tag="ksq")
            nc.vector.tensor_tensor(ksq[:], k0, k0, OP.mult)
            ksq1 = wp.tile([128, 32], f32, tag="ksq1")
            nc.vector.tensor_tensor(ksq1[:], k1, k1, OP.mult)
            mk = wp.tile([128, 32], f32, tag="mk")
            nc.vector.tensor_tensor(mk[:], ksq[:], ksq1[:], OP.add)
            nc.vector.tensor_scalar(mk[:], mk[:], 0.5, EPS, OP.mult, OP.add)
            rk = rms_r(mk, "rk")
            kn0 = wp.tile([128, 32], f32, tag="kn0")
            nc.vector.tensor_tensor(kn0[:], k0, rk[:], OP.mult)
            akm = wp.tile([128, 32], f32, tag="akm")
            nc.vector.tensor_tensor(akm[:], kn0[:], kn0[:], OP.mult)
            nc.vector.tensor_scalar(akm[:], akm[:], 0.5, EPS, OP.mult, OP.add)
            rak = rms_r(akm, "rak")
            ak = wp.tile([128, 32], f32, tag="ak")
            nc.vector.tensor_tensor(ak[:], kn0[:], rak[:], OP.mult)
            kx = wp.tile([128, 32], f32, tag="kx")
            nc.vector.tensor_tensor(kx[:], ak[:], cosk, OP.mult)
            ky = wp.tile([128, 32], f32, tag="ky")
            nc.vector.tensor_tensor(ky[:], ak[:], sink, OP.mult)

            kxs = split3(kx, 32, "kxl")
            kys = split3(ky, 32, "kyl")

            ones32 = cp.tile([128, 32], bf16, tag="ones32")
            nc.gpsimd.memset(ones32[:], 1.0)
            zeros32 = cp.tile([128, 32], bf16, tag="zeros32")
            nc.gpsimd.memset(zeros32[:], 0.0)
            Kop = cp.tile([18, 4096], bf16, tag="Kop")
            for i, bl in enumerate(KLVL):
                nc.sync.dma_start(Kop[3 * i:3 * i + 1, :], kxs[bl][:])
                nc.sync.dma_start(Kop[3 * i + 1:3 * i + 2, :], kys[bl][:])
                nc.sync.dma_start(Kop[3 * i + 2:3 * i + 3, :],
                                  ones32[:] if bl == 0 else zeros32[:])

            # ---------- v pipeline block-major ([128,32], block g) ----------
            vb0, vb1 = xkb[:, 0:32], xkb[:, 32:64]
            bsq = wp.tile([128, 32], f32, tag="bsq")
            nc.vector.tensor_tensor(bsq[:], vb0, vb0, OP.mult)
            bsq1 = wp.tile([128, 32], f32, tag="bsq1")
            nc.vector.tensor_tensor(bsq1[:], vb1, vb1, OP.mult)
            mb = wp.tile([128, 32], f32, tag="mb")
            nc.vector.tensor_tensor(mb[:], bsq[:], bsq1[:], OP.add)
            nc.vector.tensor_scalar(mb[:], mb[:], 0.5, EPS, OP.mult, OP.add)
            rb = rms_r(mb, "rb")
            xn1b = wp.tile([128, 32], f32, tag="xn1b")
            nc.vector.tensor_tensor(xn1b[:], vb1, rb[:], OP.mult)
            vv = wp.tile([128, 32], f32, tag="vv")
            nc.vector.tensor_scalar(vv[:], xn1b[:], bc[:, 2:3], None, OP.mult)
            # vbt cols per block g: [vh | vl | ones] at 3g..3g+2, all f32r
            vbt = cp.tile([128, 96], f32r, tag="vbt")
            nc.vector.tensor_copy(vbt[:, 0:96:3], vv[:])
            vlo = wp.tile([128, 32], f32, tag="vlo")
            nc.vector.tensor_tensor(vlo[:], vv[:], vbt[:, 0:96:3], OP.subtract)
            nc.vector.tensor_copy(vbt[:, 1:96:3], vlo[:])
            onesf = cp.tile([128, 32], f32, tag="onesf")
            nc.gpsimd.memset(onesf[:], 1.0)
            nc.vector.tensor_copy(vbt[:, 2:96:3], onesf[:])

            # ---------- main attention loop ----------
            # NDall: per chunk slot s, [3,512] N/D psum rows land in
            # partitions 32s..32s+31 as 16-wide col groups Nh|Nl|D.
            NDall = cp.tile([128, 48], f32, tag="NDall")
            for s in range(4):
                P = PPAIRS[s]
                nd_ps = psnd.tile([3, 512], f32)
                for j in range(P):
                    ps = pslog.tile([128, 1024], f32)
                    nc.tensor.matmul(ps[:, 0:512],
                                     Kop[:, 256 * j:256 * j + 128],
                                     Qop[:, 512 * s:512 * (s + 1)],
                                     start=True, stop=True)
                    nc.tensor.matmul(ps[:, 512:1024],
                                     Kop[:, 256 * j + 128:256 * j + 256],
                                     Qop[:, 512 * s:512 * (s + 1)],
                                     start=True, stop=True)
                    e = ep.tile([128, 1024], f32r)
                    nc.scalar.activation(e[:], ps[:], AF.Exp)
                    if j >= P - NMASK:
                        m = 4 * s + (j - (P - NMASK))
                        nc.vector.tensor_tensor(
                            e[:], e[:], masks[:, 1024 * m:1024 * (m + 1)],
                            OP.mult)
                    nc.tensor.matmul(nd_ps[:], vbt[:, 6 * j:6 * j + 3],
                                     e[:, 0:512],
                                     start=(j == 0), stop=False)
                    nc.tensor.matmul(nd_ps[:], vbt[:, 6 * j + 3:6 * j + 6],
                                     e[:, 512:1024],
                                     start=False, stop=(j == P - 1))
                nds = wp.tile([3, 512], f32, tag="nds")
                nc.vector.tensor_copy(nds[:], nd_ps[:])
                nc.sync.dma_start(NDall[32 * s:32 * s + 32, 0:16],
                                  nds[0:1, :])
                nc.sync.dma_start(NDall[32 * s:32 * s + 32, 16:32],
                                  nds[1:2, :])
                nc.sync.dma_start(NDall[32 * s:32 * s + 32, 32:48],
                                  nds[2:3, :])

            # ---------- finalize: o0, residual, gated MLP ([128,16]) -------
            Nrm = wp.tile([128, 16], f32, tag="Nrm")
            nc.vector.tensor_tensor(Nrm[:], NDall[:, 0:16], NDall[:, 16:32],
                                    OP.add)
            rD = wp.tile([128, 16], f32, tag="rD")
            nc.vector.reciprocal(rD[:], NDall[:, 32:48])
            o0 = wp.tile([128, 16], f32, tag="o0")
            nc.vector.tensor_tensor(o0[:], Nrm[:], rD[:], OP.mult)
            outt = cp.tile([128, 32], f32, tag="outt")
            h1 = wp.tile([128, 16], f32, tag="h1")
            nc.vector.tensor_tensor(h1[:], x1q, o0[:], OP.add)
            hsq0 = wp.tile([128, 16], f32, tag="hsq0")
            nc.vector.tensor_tensor(hsq0[:], x0q, x0q, OP.mult)
            hsq1 = wp.tile([128, 16], f32, tag="hsq1")
            nc.vector.tensor_tensor(hsq1[:], h1[:], h1[:], OP.mult)
            mh = wp.tile([128, 16], f32, tag="mh")
            nc.vector.tensor_tensor(mh[:], hsq0[:], hsq1[:], OP.add)
            nc.vector.tensor_scalar(mh[:], mh[:], 0.5, EPS, OP.mult, OP.add)
            rh = rms_r(mh, "rh")
            hn0 = wp.tile([128, 16], f32, tag="hn0")
            nc.vector.tensor_tensor(hn0[:], x0q, rh[:], OP.mult)
            hn1 = wp.tile([128, 16], f32, tag="hn1")
            nc.vector.tensor_tensor(hn1[:], h1[:], rh[:], OP.mult)
            g0 = wp.tile([128, 16], f32, tag="g0")
            nc.vector.tensor_scalar(g0[:], hn0[:], bc[:, 3:4], None, OP.mult)
            gt = wp.tile([128, 16], f32, tag="gt")
            nc.vector.tensor_scalar(gt[:], hn1[:], bc[:, 4:5], None, OP.mult)
            nc.vector.tensor_tensor(g0[:], g0[:], gt[:], OP.add)
            g1 = wp.tile([128, 16], f32, tag="g1")
            nc.vector.tensor_scalar(g1[:], hn0[:], bc[:, 5:6], None, OP.mult)
            nc.vector.tensor_tensor(g1[:], g1[:], gt[:], OP.add)

            def silu(g, tag):
                eg = wp.tile([128, 16], f32, tag=tag + "_e")
                nc.scalar.activation(eg[:], g[:], AF.Exp, scale=-1.0)
                nc.vector.tensor_scalar(eg[:], eg[:], 1.0, None, OP.add)
                rg = wp.tile([128, 16], f32, tag=tag + "_r")
                nc.vector.reciprocal(rg[:], eg[:])
                sg = wp.tile([128, 16], f32, tag=tag + "_s")
                nc.vector.tensor_tensor(sg[:], g[:], rg[:], OP.mult)
                return sg

            s0 = silu(g0, "s0")
            s1 = silu(g1, "s1")
            df = wp.tile([128, 16], f32, tag="df")
            nc.vector.tensor_tensor(df[:], s1[:], s0[:], OP.subtract)
            nc.vector.tensor_tensor(df[:], df[:], hn0[:], OP.mult)
            nc.vector.tensor_scalar(df[:], df[:], bc[:, 6:7], None, OP.mult)
            nc.vector.tensor_copy(outt[:, 0:16], x0q)
            nc.vector.tensor_tensor(outt[:, 16:32], h1[:], df[:], OP.add)
            nc.sync.dma_start(out_d[:], outt[:])

    nc.compile()
    return nc


def _host_inputs(x, mask, q_weight, v_weight, gate_weight, carry_weight):
    """Build the 8 per-core input maps. Host work is layout/indexing only."""
    f32 = np.float32
    x = np.ascontiguousarray(x, dtype=f32)
    theta = np.arange(L, dtype=f32) * f32(OMEGA)
    cth = np.cos(theta).astype(f32)
    sth = np.sin(theta).astype(f32)

    kk = np.arange(128)[:, None]
    qq = np.arange(512)[None, :]

    # key-side layouts shared by the two cores of a batch
    def rm(a0, a1, w):
        out = np.empty((128, 2 * w), f32)
        out[:, 0:w] = a0.reshape(128, w)
        out[:, w:2 * w] = a1.reshape(128, w)
        return out

    tk = rm(cth, sth, 32)

    in_maps = []
    for core in range(N_CORES):
        b, h = core // 2, core % 2
        chunks = CHUNKS[h]
        # query-side: local pos = 16p+m over 4 slots of 512
        qpos = np.concatenate([np.arange(512) + 512 * C for C in chunks])
        xq = rm(x[b, qpos, 0], x[b, qpos, 1], 16)
        tq = rm(cth[qpos], sth[qpos], 16)
        # key-side row-major t = 32p+n
        xk = rm(x[b, :, 0], x[b, :, 1], 32)
        # block-major: xkb[p, g] = x[128g+p]
        xkb = np.empty((128, 64), f32)
        xkb[:, 0:32] = x[b, :, 0].reshape(32, 128).T
        xkb[:, 32:64] = x[b, :, 1].reshape(32, 128).T
        # mask blob: per slot s, the last NMASK static pairs are masked.
        m = np.empty((128, 16 * 1024), np.float32)
        for s, C in enumerate(chunks):
            P = PPAIRS[s]
            for j4 in range(NMASK):
                j = P - NMASK + j4
                for side in range(2):
                    g = 2 * j + side
                    col = (4 * s + j4) * 1024 + side * 512
                    m[:, col:col + 512] = (128 * g + kk <= 512 * C + qq)
        in_maps.append({
            "xq": xq, "tq": tq, "xk": xk, "tk": tk, "xkb": xkb,
            "masks": m.astype(np.dtype("bfloat16") if False else f32),
            "wq": np.asarray(q_weight, f32),
            "wv": np.asarray(v_weight, f32),
            "wg": np.asarray(gate_weight, f32),
            "wc": np.asarray(carry_weight, f32),
        })
    # convert masks to bf16 via ml_dtypes
    import ml_dtypes
    for im in in_maps:
        im["masks"] = im["masks"].astype(ml_dtypes.bfloat16)
    return in_maps


def kernel(x, mask, q_weight, v_weight, gate_weight, carry_weight,
           _want_results=False):
    global _compiled
    from concourse.bass_utils import run_bass_kernel_spmd

    if _compiled is None:
        _compiled = _build()
    in_maps = _host_inputs(x, mask, q_weight, v_weight, gate_weight,
                           carry_weight)
    res = run_bass_kernel_spmd(_compiled, in_maps, list(range(N_CORES)))
    out = np.empty((B, L, 2), np.float32)
    for b in range(B):
        for h in range(2):
            r = res.results[2 * b + h]["out"]  # [128, 32]
            ch0 = r[:, 0:16].reshape(-1)
            ch1 = r[:, 16:32].reshape(-1)
            for s, C in enumerate(CHUNKS[h]):
                out[b, 512 * C:512 * (C + 1), 0] = ch0[512 * s:512 * (s + 1)]
                out[b, 512 * C:512 * (C + 1), 1] = ch1[512 * s:512 * (s + 1)]
    if _want_results:
        return out, res
    return out
